# revision 30
# baseline (speedup 1.0000x reference)
"""ConvLSTM (pixel-wise, 1x1 convs) Trainium2 Bass kernel.

Math (after exact algebraic folding):
  per pixel, per t:  g1 = W1x @ x_t + W1h @ h1 + b1   (W1x = Wih1 @ (W_red * denorm_scale))
                     i,f,g,o = split(g1); c1 = sig(f)*c1 + sig(i)*tanh(g); h1 = sig(o)*tanh(c1)
                     g2 = W21 @ h1 + W22 @ h2 + b2    (W21 = Wih2 @ Wc1)
                     c2,h2 analogous
  out = (W_head @ Wc2) @ h2_final + const

Sharding: batch b -> core b (8 cores, no collectives).

Layout (all-fp16 data path, fp32 only in PSUM + biases):
  S1 [92, HW]  rows 0:64 = h1, rows 64:92 = x(t) DMA'd straight from HBM
               (x is cast to fp16 on the host; no on-chip cast / staging)
  S2 [128, HW] rows 0:64 = h1 (dup via 4x-mode DVE copies), 64:128 = h2
  c1/c2 [128, HALF] fp16 : A-half pixels on partitions 0:64, B-half on 64:128
  gate planes [128, FDW] in PSUM: per-gate, A-half rows 0:64 / B-half rows
  64:128 -> every ACT/DVE pointwise op runs with all 128 partitions busy, and
  the all-fp16 operands engage the DVE 2x_1p perf mode (copies: 4x_2p).

Schedule (the ACT engine is the bottleneck at ~92% of kernel time; its
83.9M activation elements at 128 lanes / 1.2 GHz are the hard floor):
  - per t: lstm1 over all 4 column-pair groups, then lstm2 over all 4
    (groups within a pass are independent -> the in-order ACT stream stays
    saturated); gates run in two PSUM waves (i,g then f,o) of [128, 2048]
  - tanh(c)+h-writes are deferred two groups and batched ([128, 4096])
    so ACT never stalls on the DVE c-update chain
  - GpSimd never touches data (it measured ~3.5ns/elem-col on copies);
    its SWDGE queue only carries off-critical-path weight DMAs.
"""

import numpy as np

import concourse.bass as bass
import concourse.tile as tile
from concourse import bacc, mybir
from concourse.bass_utils import run_bass_kernel_spmd

F32 = mybir.dt.float32
F16 = mybir.dt.float16
AF = mybir.ActivationFunctionType

T, CIN, HID = 8, 28, 64
H = W = 128
HW = H * W            # pixels per core (one batch element)
NCORES = 8
K1, K2 = HID + CIN, 2 * HID

CHUNK = HW            # whole image resident
HALF = CHUNK // 2     # 8192
FDW = 2048            # pointwise plane free dim (pixels per half per pair)
NPAIR = HALF // FDW   # 4
NT = 512              # matmul moving tile (fp32 psum: <=512)
NSW = FDW // NT       # 4


def _fold_weights(inputs):
    """Host-side exact algebraic folding (fp32 numpy), then fp16 cast of the
    matmul operands. Biases stay fp32 (ACT bias APs)."""
    f = np.float32
    W_red = inputs["W_red"].astype(f)
    b_red = inputs["b_red"].astype(f)
    # de-normalization of channels 11 (u) and 12 (v), folded into W_red
    a = np.ones(CIN, f); a[11] = f(0.15); a[12] = f(0.12)
    d = np.zeros(CIN, f); d[11] = f(0.02); d[12] = f(-0.01)
    W_red_eff = W_red * a[None, :]
    b_red_eff = b_red + W_red @ d

    W1x = inputs["Wih1"].astype(f) @ W_red_eff          # [256, 28]
    W1h = inputs["Whh1"].astype(f)                      # [256, 64]
    b1 = (inputs["bih1"] + inputs["bhh1"]).astype(f) + inputs["Wih1"].astype(f) @ b_red_eff
    W21 = inputs["Wih2"].astype(f) @ inputs["Wc1"].astype(f)   # [256, 64]
    W22 = inputs["Whh2"].astype(f)                      # [256, 64]
    b2 = (inputs["bih2"] + inputs["bhh2"]).astype(f) + inputs["Wih2"].astype(f) @ inputs["bc1"].astype(f)
    whead = (inputs["W_head"].astype(f) @ inputs["Wc2"].astype(f))[0]     # [64]
    bhead = float((inputs["W_head"].astype(f) @ inputs["bc2"].astype(f) + inputs["b_head"].astype(f)).reshape(()))

    w1 = np.ascontiguousarray(np.concatenate([W1h, W1x], axis=1).T)  # [92, 256]: h1 rows then x rows
    w2 = np.ascontiguousarray(np.concatenate([W21, W22], axis=1).T)  # [128, 256]
    # per-gate bias vectors duplicated across the two half-planes -> [128, 4]
    bdup = lambda b: np.stack([np.concatenate([b[64 * q:64 * q + 64]] * 2) for q in range(4)], axis=1)
    wh = np.zeros((128, 1), f); wh[64:, 0] = whead
    return dict(w1=w1.astype(np.float16), w2=w2.astype(np.float16),
                b1=np.ascontiguousarray(bdup(b1)), b2=np.ascontiguousarray(bdup(b2)),
                wh=wh.astype(np.float16), bh=np.full((128, 1), bhead, f))


def build(nc):
    x_d = nc.dram_tensor("xt", [T, CIN, HW], F16, kind="ExternalInput").ap()
    w1_d = nc.dram_tensor("w1", [K1, 256], F16, kind="ExternalInput").ap()
    w2_d = nc.dram_tensor("w2", [K2, 256], F16, kind="ExternalInput").ap()
    wh_d = nc.dram_tensor("wh", [128, 1], F16, kind="ExternalInput").ap()
    b1_d = nc.dram_tensor("b1", [128, 4], F32, kind="ExternalInput").ap()
    b2_d = nc.dram_tensor("b2", [128, 4], F32, kind="ExternalInput").ap()
    bh_d = nc.dram_tensor("bh", [128, 1], F32, kind="ExternalInput").ap()
    # out[i, j] = pixel j*128 + i of this core's [H, W] map (host transposes)
    out_d = nc.dram_tensor("out", [128, HW // 128], F32, kind="ExternalOutput").ap()

    with tile.TileContext(nc) as tc:
        with (
            tc.tile_pool(name="const", bufs=1) as const,
            tc.tile_pool(name="state", bufs=1) as state,
            tc.tile_pool(name="planes", bufs=3) as planes,
            tc.tile_pool(name="outp", bufs=1) as outp,
            tc.tile_pool(name="psum", bufs=1, space=bass.MemorySpace.PSUM) as psum,
        ):
            S1 = state.tile([K1, CHUNK], F16, tag="S1")
            S2 = state.tile([K2, CHUNK], F16, tag="S2")
            c1 = state.tile([128, HALF], F16, tag="c1")
            c2 = state.tile([128, HALF], F16, tag="c2")

            # Startup critical path: lstm1 weights + bias + x(0) halves.
            # scalar HWDGE queue carries w1/b1 first (tiny) then the B-half
            # x strips; sync HWDGE carries the A-half strips; everything not
            # needed in the first pair rides the (slow-init) gpsimd SWDGE.
            w1f = const.tile([K1, 256], F16, tag="w1f")
            w2f = const.tile([K2, 256], F16, tag="w2f")
            whf = const.tile([128, 1], F16, tag="whf")
            b1_sb = const.tile([128, 4], F32, tag="b1")
            b2_sb = const.tile([128, 4], F32, tag="b2")
            bh_sb = const.tile([128, 1], F32, tag="bh")
            # pair 0 needs exactly cols [0:FDW) + [HALF:HALF+FDW): land those
            # short pieces first on both queues, then w1/b1, then the rest.
            nc.sync.dma_start(S1[HID:K1, 0:FDW], x_d[0][:, 0:FDW])
            nc.scalar.dma_start(S1[HID:K1, HALF:HALF + FDW], x_d[0][:, HALF:HALF + FDW])
            nc.sync.dma_start(S1[HID:K1, FDW:HALF], x_d[0][:, FDW:HALF])
            nc.scalar.dma_start(w1f[:], w1_d)
            nc.scalar.dma_start(b1_sb[:], b1_d)
            nc.scalar.dma_start(S1[HID:K1, HALF + FDW:CHUNK], x_d[0][:, HALF + FDW:CHUNK])
            nc.gpsimd.dma_start(w2f[:], w2_d)
            nc.gpsimd.dma_start(whf[:], wh_d)
            nc.gpsimd.dma_start(b2_sb[:], b2_d)
            nc.gpsimd.dma_start(bh_sb[:], bh_d)
            # Stage weights via a DMA'd tile + one convert copy each, so every
            # matmul waits on a single compute producer (the fused LDWEIGHTS
            # has very few sync-wait slots; direct multi-queue DMA deps
            # overflow it -> walrus "Too many sync wait commands").
            w1_sb = const.tile([K1, 256], F16, tag="w1")
            w2_sb = const.tile([K2, 256], F16, tag="w2")
            wh_sb = const.tile([128, 1], F16, tag="wh")
            nc.vector.tensor_copy(w1_sb[:], w1f[:])
            nc.vector.tensor_copy(w2_sb[:], w2f[:])
            nc.vector.tensor_copy(wh_sb[:], whf[:])

            out_sb = outp.tile([128, HW // 128], F32, tag="osb")

            # Deferred (tanh-c + h-write) blocks, flushed two pairs later so
            # the in-order ACT stream never stalls on the DVE c-update chain
            # and tanh-c runs once per pair-pair ([128, 2*FDW], less per-instr
            # overhead on the bottleneck ACT engine).
            pend = []

            def flush_pend():
                if not pend:
                    return
                entries = list(pend)
                pend.clear()
                cc = entries[0][2]
                p0 = entries[0][1]
                n = len(entries)
                cgm = slice(p0 * FDW, (p0 + n) * FDW)
                tch = planes.tile([128, 2 * FDW], F16, tag="tc")
                nc.scalar.activation(tch[0:128, 0:n * FDW], cc[:, cgm], AF.Tanh)
                for k, (lst, p, _cc, so, cps) in enumerate(entries):
                    a0 = p * FDW
                    b0 = HALF + p * FDW
                    tA = tch[0:64, k * FDW:(k + 1) * FDW]
                    tB = tch[64:128, k * FDW:(k + 1) * FDW]
                    if lst == 0:
                        # h1 -> S1 rows 0:64 directly (2x TT), dup into S2
                        # rows 0:64 via 4x-mode fp16 copies
                        nc.vector.tensor_mul(S1[0:HID, a0:a0 + FDW], so[0:64, :], tA)
                        nc.vector.tensor_mul(S1[0:HID, b0:b0 + FDW], so[64:128, :], tB)
                        if cps:
                            nc.vector.tensor_copy(S2[0:HID, a0:a0 + FDW], S1[0:HID, a0:a0 + FDW])
                            nc.vector.tensor_copy(S2[0:HID, b0:b0 + FDW], S1[0:HID, b0:b0 + FDW])
                    else:
                        nc.vector.tensor_mul(S2[HID:K2, a0:a0 + FDW], so[0:64, :], tA)
                        nc.vector.tensor_mul(S2[HID:K2, b0:b0 + FDW], so[64:128, :], tB)

            def maybe_flush(eager=False):
                if len(pend) == 2 or (eager and pend):
                    flush_pend()

            def gate_mms(Pt, w_sb, SS, ks, q, a0, b0):
                # one gate's matmuls over an FDW-wide pair of column groups
                # (s-major order: alternating the PSUM half each instruction
                # measured ~4us faster than half-major on hardware)
                for s in range(NSW):
                    for (cb, po) in ((a0, 0), (b0, 64)):
                        nc.tensor.matmul(
                            Pt[po:po + 64, s * NT:(s + 1) * NT],
                            w_sb[ks, q * 64:(q + 1) * 64],
                            SS[ks, cb + s * NT:cb + (s + 1) * NT],
                        )

            for t in range(T):
                # x(t) straight into S1's x rows (fp16 in HBM, no convert);
                # x(0) was issued above, ahead of the weight DMAs.
                if t > 0:
                    nc.sync.dma_start(S1[HID:K1, :], x_d[t])

                # pass 0: lstm1 over all pairs; pass 1: lstm2 over all pairs
                # (pairs within a pass are independent -> ACT stays saturated)
                for lst in (0, 1):
                    if lst == 0:
                        w_sb, b_sb, SS = w1_sb, b1_sb, S1
                        ks = slice(0, K1) if t > 0 else slice(HID, K1)
                        cc = c1
                    else:
                        w_sb, b_sb = w2_sb, b2_sb
                        # t=0: only the h1 half of S2 would be live, and it is
                        # identical to S1 rows 0:64 -> read S1 directly (the
                        # t=0 h1->S2 copies are then dead and skipped below)
                        SS = S2 if t > 0 else S1
                        ks = slice(0, K2) if t > 0 else slice(0, HID)
                        cc = c2

                    for p in range(NPAIR):
                        a0 = p * FDW           # A-half cols in S1/S2
                        b0 = HALF + p * FDW    # B-half cols
                        cg = slice(p * FDW, (p + 1) * FDW)

                        # wave 1: gates i (q=0) and g (q=2), PSUM tags Pi/Pg
                        Pi = psum.tile([128, FDW], F32, tag="Pi", name="Pi")
                        Pg = psum.tile([128, FDW], F32, tag="Pg", name="Pg")
                        gate_mms(Pi, w_sb, SS, ks, 0, a0, b0)
                        gate_mms(Pg, w_sb, SS, ks, 2, a0, b0)
                        si = planes.tile([128, FDW], F16, tag="si")
                        tg = planes.tile([128, FDW], F16, tag="tg")
                        nc.scalar.activation(si[:], Pi[:], AF.Sigmoid, bias=b_sb[:, 0:1])
                        nc.scalar.activation(tg[:], Pg[:], AF.Tanh, bias=b_sb[:, 2:3])

                        if t > 0:
                            t2 = planes.tile([128, FDW], F16, tag="t2")
                            nc.vector.tensor_mul(t2[:], si[:], tg[:])
                            # wave 2: gates f (q=1) and o (q=3) reuse the banks
                            Pf = psum.tile([128, FDW], F32, tag="Pi", name="Pf")
                            Po = psum.tile([128, FDW], F32, tag="Pg", name="Po")
                            gate_mms(Pf, w_sb, SS, ks, 1, a0, b0)
                            gate_mms(Po, w_sb, SS, ks, 3, a0, b0)
                            sf = planes.tile([128, FDW], F16, tag="sf")
                            so = planes.tile([128, FDW], F16, tag="so")
                            nc.scalar.activation(sf[:], Pf[:], AF.Sigmoid, bias=b_sb[:, 1:2])
                            nc.scalar.activation(so[:], Po[:], AF.Sigmoid, bias=b_sb[:, 3:4])
                            t1 = planes.tile([128, FDW], F16, tag="t1")
                            nc.vector.tensor_mul(t1[:], sf[:], cc[:, cg])
                            maybe_flush(eager=(t == T - 1 and lst == 1))
                            nc.vector.tensor_add(cc[:, cg], t1[:], t2[:])
                        else:
                            # t=0: c_prev = 0, forget gate unused -> c = si*tg.
                            # o-gate MMs go in tag Pi (freed early by the si
                            # read) so they overlap the tanh-g ACT instr.
                            nc.vector.tensor_mul(cc[:, cg], si[:], tg[:])
                            Po = psum.tile([128, FDW], F32, tag="Pi", name="Po")
                            gate_mms(Po, w_sb, SS, ks, 3, a0, b0)
                            so = planes.tile([128, FDW], F16, tag="so")
                            nc.scalar.activation(so[:], Po[:], AF.Sigmoid, bias=b_sb[:, 3:4])
                            maybe_flush()
                        pend.append((lst, p, cc, so, t > 0))

            flush_pend()

            # head: out[pix] = whead @ h2[pix] + bh, pixels as matmul M-dim
            ncols = HW // 128
            ph = psum.tile([128, FDW], F32, tag="Pi", name="ph")
            for j in range(ncols):
                nc.tensor.matmul(
                    ph[:, j:j + 1],
                    S2[HID:K2, j * 128:(j + 1) * 128],
                    wh_sb[64:128, 0:1],
                )
            nc.vector.tensor_scalar_add(out_sb[:], ph[:, 0:ncols], bh_sb[:, 0:1])

            nc.sync.dma_start(out_d, out_sb[:])
    nc.compile()
    return nc


def _make_nc():
    # Bacc (not raw Bass): its compile() runs move_matmul_waits_to_ldweights +
    # generate_event_semaphores, required to satisfy TRN2's 1-wait-per-inst limit.
    return bacc.Bacc("TRN2", target_bir_lowering=False, debug=False,
                     num_devices=NCORES, enable_partition_id=False)


def _in_maps(inputs):
    folded = _fold_weights(inputs)
    x = np.asarray(inputs["x"])
    maps = []
    for b in range(NCORES):
        m = dict(folded)
        m["xt"] = np.ascontiguousarray(
            x[b].reshape(T, CIN, HW).astype(np.float16))
        maps.append(m)
    return maps


def _assemble(results):
    out = np.empty((NCORES, H, W), np.float32)
    for b in range(NCORES):
        o = results[b]["out"]          # [128, HW//128], o[i, j] = pixel j*128+i
        out[b] = o.T.reshape(H, W)
    return out


def _run(inputs, trace=False):
    nc = build(_make_nc())
    maps = _in_maps(inputs)
    res = run_bass_kernel_spmd(nc, maps, core_ids=list(range(NCORES)), trace=trace)
    return _assemble(res.results), res


def kernel(**inputs) -> np.ndarray:
    out, _ = _run(inputs, trace=False)
    return out

